# revision 1
# baseline (speedup 1.0000x reference)
"""Trainium2 Bass kernel for a full-attention MHA layer (B=2, S=2048, HID=2048,
16 heads, head_dim=128, RoPE, no mask), sharded over 8 NeuronCores as
2 batches x 4 head-groups (4 heads per core).

Per-core dataflow (feature-major, so no probability transposes are needed):
  hiddenT  = PE-transpose(hidden)                  [k,t]  (bf16, on-chip)
  qT,kT    = w_qkvT.T @ hiddenT                    [d,t]  per head + RoPE
  v        = hiddenT.T @ w_vT                      [t,d]  natural layout
  ST       = kT.T @ qT                             [tk,tq] scores transposed
  PT       = exp(ST * scale)                       (ACT, PSUM->SBUF fused)
  den      = onesT.T @ PT                          partition sum, replicated
  OT       = (v.T @ PT) * (1/den)                  [d,tq]
  OUT      = OT.T @ w_oT                           [t,o]  partial over heads

Host sums the 4 per-batch partial OUTs.
"""
import numpy as np
import ml_dtypes

import concourse.bass as bass
import concourse.mybir as mybir
from concourse import bacc, tile

B, S, HID = 2, 2048, 2048
NH, HD = 16, 128
G = 4                 # head-groups = cores per batch
NHL = NH // G         # heads per core
KO = HID // 128       # 16 contraction chunks
TS = 512              # token slice for the projection phase
NSL = S // TS         # 8
TQ = 512              # query-tile width in attention
NTQ = S // TQ         # 4
NTK = S // 128        # 16 key chunks
FQK = NHL * HD        # 512 features for q (and k) per core
FV = NHL * HD         # 512 features for v per core
BF16 = mybir.dt.bfloat16
F32 = mybir.dt.float32
SCALE = 1.0 / float(np.sqrt(HD))

N_CORES = 8


def _emit(nc, tc, hid, wq, wo, cosT, ssinT, outp, repeats=1):
    from contextlib import ExitStack
    ctx = ExitStack()
    with ctx:
        const = ctx.enter_context(tc.tile_pool(name="const", bufs=1))
        persist = ctx.enter_context(tc.tile_pool(name="persist", bufs=1))
        work = ctx.enter_context(tc.tile_pool(name="work", bufs=2))
        small = ctx.enter_context(tc.tile_pool(name="small", bufs=2))
        psA = ctx.enter_context(tc.tile_pool(name="psA", bufs=5, space="PSUM"))
        psB = ctx.enter_context(tc.tile_pool(name="psB", bufs=2, space="PSUM"))
        psC = ctx.enter_context(tc.tile_pool(name="psC", bufs=1, space="PSUM"))

        # ---- constants (SWDGE queues so they don't block the hid stream) ----
        ones_sb = const.tile([128, 128], BF16)
        nc.vector.memset(ones_sb, 1.0)
        wq_full = const.tile([128, KO, 3 * FQK], BF16)
        for ko in range(KO):
            nc.gpsimd.dma_start(wq_full[:, ko, :], wq[:, ko, :])
        wq_sbs = [wq_full[:, ko, :] for ko in range(KO)]
        cos_sb = const.tile([128, S], BF16)
        nc.gpsimd.dma_start(cos_sb, cosT)
        ssin_sb = const.tile([128, S], BF16)
        nc.gpsimd.dma_start(ssin_sb, ssinT)
        wo_sb = const.tile([128, NHL, HID], BF16)
        nc.gpsimd.dma_start(wo_sb, wo)

        for _rep in range(repeats):
            _emit_body(nc, tc, hid, outp, wq_sbs, wo_sb, cos_sb, ssin_sb,
                       ones_sb, persist, work, small, psA, psB, psC)


def _emit_body(nc, tc, hid, outp, wq_sbs, wo_sb, cos_sb, ssin_sb,
               ones_sb, persist, work, small, psA, psB, psC):
        # ---- persistent activations ----
        qT = persist.tile([128, NHL, S], BF16, tag="qT", bufs=1)   # [d, h, t]
        kT = persist.tile([128, NHL, S], BF16, tag="kT", bufs=1)   # [d, h, t]
        vN = persist.tile([128, NTK, FV], BF16, tag="vN", bufs=1)  # [t%128, t//128, f]
        oT = persist.tile([128, NHL, S], BF16, tag="oT", bufs=1)   # [d, h, tq]

        # ======== Phase A+B: hiddenT + QKV projections + RoPE ========
        def emit_slice_transposes(ts_i):
            t0 = ts_i * TS
            hT = work.tile([128, KO, TS], BF16, tag="hT", name="hT")
            for tt in range(TS // 128):
                hid_sb = work.tile([128, HID], F32, tag="hid", bufs=2, name="hid_sb")
                for q4 in range(4):
                    nc.sync.dma_start(hid_sb[:, q4 * 512:(q4 + 1) * 512],
                                      hid[t0 + tt * 128: t0 + (tt + 1) * 128, q4 * 512:(q4 + 1) * 512])
                hidb = work.tile([128, HID], BF16, tag="hidb", bufs=1, name="hidb")
                nc.vector.tensor_copy(hidb, hid_sb)
                nc.sync.dma_start_transpose(hT[:, :, tt * 128:(tt + 1) * 128], hidb)
            return hT

        for ts_i in range(NSL):
            t0 = ts_i * TS
            hT = emit_slice_transposes(ts_i)
            # q,k projections: f-tile = one head's 128 dims (0-3 q, 4-7 k)
            for ft in range(2 * NHL):
                ps = psA.tile([128, 512], F32, tag="mm")
                for ko in range(KO):
                    nc.tensor.matmul(ps[:, :TS],
                                     wq_sbs[ko][:, ft * 128:(ft + 1) * 128],
                                     hT[:, ko, :],
                                     start=(ko == 0), stop=(ko == KO - 1))
                if ft < NHL:
                    dest = qT[:, ft, t0:t0 + TS]
                    dlo = qT[0:64, ft, t0:t0 + TS]
                    dhi = qT[64:128, ft, t0:t0 + TS]
                else:
                    dest = kT[:, ft - NHL, t0:t0 + TS]
                    dlo = kT[0:64, ft - NHL, t0:t0 + TS]
                    dhi = kT[64:128, ft - NHL, t0:t0 + TS]
                nc.vector.tensor_copy(dest, ps[:, :TS])
                # RoPE: dest = dest*cos + swap(dest)*ssin  (sign folded into ssin)
                sw = small.tile([128, TS], BF16, tag="sw", bufs=2)
                nc.scalar.dma_start(sw[0:64, :], dhi)
                nc.scalar.dma_start(sw[64:128, :], dlo)
                nc.vector.tensor_mul(dest, dest, cos_sb[:, t0:t0 + TS])
                nc.vector.tensor_mul(sw, sw, ssin_sb[:, t0:t0 + TS])
                nc.vector.tensor_add(dest, dest, sw)
            # v projection in natural [t, f] layout
            for tt in range(TS // 128):
                ps = psA.tile([128, 512], F32, tag="mm")
                for ko in range(KO):
                    nc.tensor.matmul(ps,
                                     hT[:, ko, tt * 128:(tt + 1) * 128],
                                     wq_sbs[ko][:, 2 * FQK:3 * FQK],
                                     start=(ko == 0), stop=(ko == KO - 1))
                nc.vector.tensor_copy(vN[:, ts_i * (TS // 128) + tt, :], ps)

        # ======== Phase C+D: attention, with out-proj interleaved per tq ========
        for tqi in range(NTQ):
            tq0 = tqi * TQ
            for h in range(NHL):
                den = psB.tile([128, TQ], F32, tag="acc")
                pv = psB.tile([128, TQ], F32, tag="acc")

                def emit_score(tkc):
                    ps = psA.tile([128, 512], F32, tag="mm", name="ps")
                    nc.tensor.matmul(ps[:, :TQ],
                                     kT[:, h, tkc * 128:(tkc + 1) * 128],
                                     qT[:, h, tq0:tq0 + TQ],
                                     start=True, stop=True)
                    pt = small.tile([128, TQ], BF16, tag="pt", bufs=6, name="pt")
                    nc.scalar.activation(pt, ps[:, :TQ],
                                         mybir.ActivationFunctionType.Exp,
                                         scale=SCALE)
                    return pt

                # software pipeline: scores run ~4 chunks ahead; the den
                # reduction collapses each group of 4 prob chunks to one
                # matmul via bf16 pair-sums on DVE (final accumulation of
                # the 4 group partials stays in fp32 PSUM).
                pts = [emit_score(c) for c in range(4)]
                for grp in range(NTK // 4):
                    g0 = grp * 4
                    cur = [pts[(g0 + j) % 4] for j in range(4)]
                    for j in range(4):
                        nc.tensor.matmul(pv,
                                         vN[:, g0 + j, h * HD:(h + 1) * HD],
                                         cur[j],
                                         start=(g0 + j == 0), stop=(g0 + j == NTK - 1))
                        if g0 + j + 4 < NTK:
                            pts[(g0 + j) % 4] = emit_score(g0 + j + 4)
                    s1 = small.tile([128, TQ], BF16, tag="ptsum", bufs=2, name="s1")
                    nc.vector.tensor_add(s1, cur[0], cur[1])
                    s2 = small.tile([128, TQ], BF16, tag="ptsum", bufs=2, name="s2")
                    nc.vector.tensor_add(s2, cur[2], cur[3])
                    nc.vector.tensor_add(s1, s1, s2)
                    nc.tensor.matmul(den, ones_sb, s1,
                                     start=(grp == 0), stop=(grp == NTK // 4 - 1))
                rec = small.tile([128, TQ], F32, tag="rec", bufs=1)
                nc.vector.reciprocal(rec, den)
                nc.vector.tensor_mul(oT[:, h, tq0:tq0 + TQ], pv, rec)
            # out-projection for the t-range covered by this tq tile
            for tt in range(tqi * (TQ // 128), (tqi + 1) * (TQ // 128)):
                for ot in range(HID // 512):
                    ps = psC.tile([128, 512], F32, tag="out")
                    for h in range(NHL):
                        nc.tensor.matmul(ps,
                                         oT[:, h, tt * 128:(tt + 1) * 128],
                                         wo_sb[:, h, ot * 512:(ot + 1) * 512],
                                         start=(h == 0), stop=(h == NHL - 1))
                    ob = small.tile([128, 512], F32, tag="ob", bufs=2)
                    if (tt + ot) % 2 == 0:
                        nc.vector.tensor_copy(ob, ps)
                    else:
                        nc.scalar.copy(ob, ps)
                    nc.sync.dma_start(outp[tt * 128:(tt + 1) * 128, ot * 512:(ot + 1) * 512], ob)


def build(repeats=1):
    nc = bacc.Bacc("TRN2", target_bir_lowering=False, debug=False)
    hid = nc.dram_tensor("hid", [S, HID], F32, kind="ExternalInput")
    wq = nc.dram_tensor("wq", [128, KO, 3 * FQK], BF16, kind="ExternalInput")
    wo = nc.dram_tensor("wo", [128, NHL, HID], BF16, kind="ExternalInput")
    cosT = nc.dram_tensor("cosT", [128, S], BF16, kind="ExternalInput")
    ssinT = nc.dram_tensor("ssinT", [128, S], BF16, kind="ExternalInput")
    outp = nc.dram_tensor("outp", [S, HID], F32, kind="ExternalOutput")
    with tile.TileContext(nc) as tc:
        _emit(nc, tc, hid.ap(), wq.ap(), wo.ap(), cosT.ap(), ssinT.ap(), outp.ap(),
              repeats=repeats)
    nc.compile()
    return nc


def shard_inputs(hidden_states, cos, sin, w_qkv, w_o):
    """Build the 8 per-core input maps (host-side layout prep)."""
    hidden_states = np.asarray(hidden_states, dtype=np.float32)
    cos = np.asarray(cos, dtype=np.float32)
    sin = np.asarray(sin, dtype=np.float32)
    w_qkv = np.asarray(w_qkv, dtype=np.float32)
    w_o = np.asarray(w_o, dtype=np.float32)

    cosT = np.ascontiguousarray(cos[:, 0, :].T).astype(ml_dtypes.bfloat16)
    sT = sin[:, 0, :].T.copy()
    sT[:64] = -sT[:64]
    ssinT = np.ascontiguousarray(sT).astype(ml_dtypes.bfloat16)

    woT = w_o.T  # [j, o]
    in_maps = []
    for c in range(N_CORES):
        b, g = divmod(c, G)
        rows = np.concatenate([
            w_qkv[FQK * g: FQK * (g + 1)],
            w_qkv[NH * HD + FQK * g: NH * HD + FQK * (g + 1)],
            w_qkv[2 * NH * HD + FQK * g: 2 * NH * HD + FQK * (g + 1)],
        ], axis=0)                                   # [1536, 2048]
        wq_pack = np.ascontiguousarray(
            rows.T.reshape(KO, 128, 3 * FQK).transpose(1, 0, 2)
        ).astype(ml_dtypes.bfloat16)                 # [128, KO, 1536]
        wo_pack = np.ascontiguousarray(
            woT[FQK * g: FQK * (g + 1)].reshape(NHL, 128, HID).transpose(1, 0, 2)
        ).astype(ml_dtypes.bfloat16)                 # [128, NHL, 2048]
        in_maps.append({
            "hid": np.ascontiguousarray(hidden_states[b]),
            "wq": wq_pack,
            "wo": wo_pack,
            "cosT": cosT,
            "ssinT": ssinT,
        })
    return in_maps


def gather_outputs(results):
    """results: list of 8 dicts with 'outp' -> full [B, S, HID] output."""
    out = np.zeros((B, S, HID), dtype=np.float32)
    for c in range(N_CORES):
        b = c // G
        out[b] += results[c]["outp"]
    return out


# ---------------- cached runner over PJRT/axon ----------------
_RUNNER = None


def _make_runner():
    import jax
    from jax.sharding import Mesh, PartitionSpec, NamedSharding
    from jax.experimental.shard_map import shard_map
    from concourse import bass2jax

    nc = build()
    bass2jax.install_neuronx_cc_hook()
    partition_name = nc.partition_id_tensor.name if nc.partition_id_tensor else None
    in_names, out_names, out_avals = [], [], []
    for alloc in nc.m.functions[0].allocations:
        if not isinstance(alloc, mybir.MemoryLocationSet):
            continue
        name = alloc.memorylocations[0].name
        if alloc.kind == "ExternalInput":
            if name != partition_name:
                in_names.append(name)
        elif alloc.kind == "ExternalOutput":
            out_names.append(name)
            out_avals.append(jax.core.ShapedArray(
                tuple(alloc.tensor_shape), mybir.dt.np(alloc.dtype)))
    n_params = len(in_names)
    all_in_names = list(in_names) + list(out_names)
    if partition_name is not None:
        all_in_names.append(partition_name)

    import hashlib
    import os as _os
    _tag = hashlib.sha256(open(__file__, "rb").read()
                          + _os.environ.get("BASS_KERNEL_TAG", "").encode()).hexdigest()[:12]

    def _body(*args):
        operands = list(args)
        if partition_name is not None:
            operands.append(bass2jax.partition_id_tensor())
        outs = bass2jax._bass_exec_p.bind(
            *operands,
            out_avals=tuple(out_avals),
            in_names=tuple(all_in_names),
            out_names=tuple(out_names),
            lowering_input_output_aliases=(),
            sim_require_finite=True,
            sim_require_nnan=True,
            nc=nc,
        )
        return tuple(outs)

    devices = jax.devices()[:N_CORES]
    mesh = Mesh(np.asarray(devices), ("core",))
    n_outs = len(out_names)
    in_specs = (PartitionSpec("core"),) * (n_params + n_outs)
    out_specs = (PartitionSpec("core"),) * n_outs
    donate = tuple(range(n_params, n_params + n_outs))
    _body.__name__ = f"body_{_tag}"
    _sharded = shard_map(_body, mesh=mesh, in_specs=in_specs, out_specs=out_specs,
                         check_rep=False)

    def _entry(*args):
        return _sharded(*args)
    _entry.__name__ = f"bass_attn_{_tag}"
    fn = jax.jit(_entry, donate_argnums=donate, keep_unused=True)
    sharding = NamedSharding(mesh, PartitionSpec("core"))

    class Runner:
        def __init__(self):
            self.fn = fn
            self.nc = nc
            self.in_names = in_names
            self.out_names = out_names
            self.out_avals = out_avals
            self.sharding = sharding

        def stage(self, in_maps):
            import jax
            concat = [np.concatenate([in_maps[c][n] for c in range(N_CORES)], axis=0)
                      for n in self.in_names]
            return [jax.device_put(x, self.sharding) for x in concat]

        def zeros(self):
            import jax
            import jax.numpy as jnp
            if not hasattr(self, "_zeros_fn"):
                shapes = [((N_CORES * av.shape[0],) + tuple(av.shape[1:]), av.dtype)
                          for av in self.out_avals]
                self._zeros_fn = jax.jit(
                    lambda: tuple(jnp.zeros(s, d) for s, d in shapes),
                    out_shardings=tuple(self.sharding for _ in shapes))
            return list(self._zeros_fn())

        def run(self, dev_in, outs=None):
            if outs is None:
                outs = self.zeros()
            return self.fn(*dev_in, *outs)

        def split(self, outs):
            import jax
            jax.block_until_ready(outs)
            res = []
            for c in range(N_CORES):
                res.append({
                    n: np.asarray(outs[i]).reshape(
                        N_CORES, *self.out_avals[i].shape)[c]
                    for i, n in enumerate(self.out_names)})
            return res

    return Runner()


def get_runner():
    global _RUNNER
    if _RUNNER is None:
        _RUNNER = _make_runner()
    return _RUNNER


def kernel(hidden_states, cos, sin, w_qkv, w_o):
    r = get_runner()
    in_maps = shard_inputs(hidden_states, cos, sin, w_qkv, w_o)
    dev_in = r.stage(in_maps)
    outs = r.run(dev_in)
    results = r.split(outs)
    return gather_outputs(results)

